# revision 74
# baseline (speedup 1.0000x reference)
"""Trainium2 Bass kernel: GAT-style attention layer, data-parallel over 8 NeuronCores.

Reference computation (per node n, K=32 neighbors, D=128 features, L=64 labels):
    h     = lrelu(x @ W)                  [N,K,D]
    e     = lrelu(h @ v + bias)           [N,K,1]
    alpha = softmax_k(e)                  [N,K]
    out   = sum_k alpha[n,k] * labels[n,k,:]   [N,L]

Sharding: pure data parallel over nodes (6250/core, zero-padded to 6400).
The kernel is HBM-bandwidth-bound, so the host pre-quantizes the big inputs:
x to fp8-e3m4 (4-mantissa-bit fp8; feeds the PE directly — fp8 matmul runs at
bf16 speed), labels to bf16, output returned as bf16 and upcast on the host.
Per-core DRAM traffic drops 155MB -> 52MB; rel err stays ~2e-3 (vs 2e-2 gate).

Device pipeline per 256-node tile (software-pipelined by one tile):
  mm1   z^T[e,(k,n)] = W^T @ x^T          TensorE, bf16 W x fp8 x, 2x512-col
        matmuls per 1024-col PSUM chunk
  lrelu PSUM->SBUF bf16, split across ScalarE (full Prelu) and VectorE
        (relu-only, one tensor_scalar_max op; the 0.2z part of those chunks'
        scores is restored exactly inside mm2 — see ACT_CHUNKS comment) so
        the 8192-elem/partition/tile pass doesn't bottleneck either engine
  mm2   S[128,TN] = per-k selector matmuls, 4-way column-tiled: group j=k%4
        writes PSUM partitions 32j..32j+31 (4 concurrent matmuls on PE),
        plus a 0.2*(W@v)^T x correction matmul for relu-only chunks
  e=lrelu(S+bias128), w=exp(e)            ScalarE; bias128 is the host-
        expanded per-partition bias (row 32j+i holds bias[4i+j])
  w^T   TensorE transpose [128,128] -> [n, kperm]; row sums over the 32 used
        columns (strided view) via ScalarE accum_out
  agg   out[n,l] = sum_k wT*lab on VectorE with UNNORMALIZED exp-weights
        (softmax 1/sum is applied once per node on the final [128, L] slice,
        keeping the normalization off the critical path); labels are
        host-laid l-major with kperm innermost (col 8j+i = original k=4i+j)
        so the broadcast multiply and the k-reduction tree are unit-stride
        and run in the DVE 16-bit 2x mode
  out   bf16 store on the sync (HWDGE) queue; host upcasts to f32
"""
import sys

sys.path.insert(0, "/opt/trn_rl_repo")
import numpy as np

N, K, D, L = 50000, 32, 128, 64
NEG = 0.2
NCORES = 8
NPER = N // NCORES          # 6250
TN = 256                    # nodes per tile
NSUB = TN // 128            # sub-tiles of 128 nodes
NPAD = 6400                 # padded nodes per core
NT = NPAD // TN             # 25 tiles
# lrelu chunks on ScalarE (full Prelu); the rest on VectorE, which computes
# only relu(z) in one op (the walrus verifier rejects the 3-input
# scalar_tensor_tensor form from PSUM). The missing 0.2*z part of those
# chunks' scores is restored exactly in mm2: sum_c v_c*lrelu(z_c) =
# 0.8*sum_c v_c*relu(z_c) + 0.2*(W@v)^T x, with 0.2*W@v host-precomputed.
ACT_CHUNKS = (0, 2, 4, 5, 6, 7)
DVE_CHUNKS = tuple(c for c in range(8) if c not in ACT_CHUNKS)

# kperm: alpha/label column c=8j+i corresponds to original neighbor k=4i+j
KPERM = [4 * (c % 8) + c // 8 for c in range(K)]

LAST_RESULT = None
_cache = {}


def build(nt):
    import concourse.bass as bass
    import concourse.tile as tile
    from concourse import bacc, mybir

    f32 = mybir.dt.float32
    bf16 = mybir.dt.bfloat16
    f8 = mybir.dt.float8e3
    AF = mybir.ActivationFunctionType
    OP = mybir.AluOpType
    PSUM = bass.MemorySpace.PSUM

    nc = bacc.Bacc(
        "TRN2", target_bir_lowering=False, debug=False, num_devices=NCORES
    )
    # weights arrive pre-cast to bf16 from the host so every load is a plain
    # HWDGE copy on the sync queue (no SWDGE cast in the startup chain)
    x_ext = nc.declare_dram_parameter("x", [nt, 128, K * TN], f8, False)
    lab_ext = nc.declare_dram_parameter("lab", [nt, 128, NSUB * L * K], bf16, False)
    w_ext = nc.declare_dram_parameter("w", [D, D], bf16, False)
    v_ext = nc.declare_dram_parameter("v", [D, 1], bf16, False)
    b_ext = nc.declare_dram_parameter("b", [128, 1], f32, False)
    wv_ext = nc.declare_dram_parameter("wv", [D, 1], bf16, False)  # 0.2*(W@v)
    out_ext = nc.declare_dram_parameter("out", [nt, 128, NSUB * L], bf16, isOutput=True)

    with tile.TileContext(nc) as tc:
        with (
            tc.tile_pool(name="const", bufs=1) as const,
            tc.tile_pool(name="xp", bufs=3) as xp,
            tc.tile_pool(name="labp", bufs=3) as labp,
            tc.tile_pool(name="hp", bufs=2) as hp,
            tc.tile_pool(name="wp", bufs=2) as wp,
            tc.tile_pool(name="smallp", bufs=4) as smallp,
            tc.tile_pool(name="dkp", bufs=2) as dkp,
            tc.tile_pool(name="outp", bufs=2) as outp,
            tc.tile_pool(name="zps", bufs=3, space=PSUM) as zps,
            tc.tile_pool(name="sps", bufs=1, space=PSUM) as sps,
            tc.tile_pool(name="wtps", bufs=1, space=PSUM) as wtps,
        ):
            W_sb = const.tile([128, 128], bf16)
            nc.sync.dma_start(W_sb[:], w_ext[:])
            v_sb = const.tile([128, 1], bf16)
            nc.sync.dma_start(v_sb[:], v_ext[:])
            wv_sb = const.tile([128, 1], bf16)
            nc.sync.dma_start(wv_sb[:], wv_ext[:])
            bias_sb = const.tile([128, 1], f32)
            nc.sync.dma_start(bias_sb[:], b_ext[:])
            ones = const.tile([128, 128], bf16)
            nc.vector.memset(ones[:], 1.0)
            mask = const.tile([128, 128], bf16)         # identity matrix
            nc.gpsimd.affine_select(
                mask[:], ones[:], pattern=[[1, 128]],
                compare_op=OP.is_equal, fill=0.0, base=0, channel_multiplier=-1,
            )

            # PE warmup burst: dense dummy matmuls (only W/ones deps, both
            # ready ~1us in) while the first x tile loads. The HAM clock gate
            # needs ~3.4us of sustained PE activity to open (4096 cycles at
            # the cold 1.2 GHz), so 32 x 128-col matmuls ~= 3.4us cold.
            warm_ps = zps.tile([128, 512], f32, name="warm_ps", tag="z")
            for _ in range(32):
                nc.tensor.matmul(
                    warm_ps[:, 0:128], W_sb[:], ones[:], skip_group_check=True
                )

            # vks block k=4i+j ([128,32] at cols 32k..32k+31) holds v in column
            # i only, so the k-th score matmul (col-group j) writes PSUM
            # partition 32j+i of S. Flat column = 32k+i = 129i + 32j.
            # Within group j, column i corresponds to mm1 chunk i (k//4 = i):
            # DVE chunks carry relu(z), so their selector is scaled by 0.8 and
            # the 0.2*(W@v)^T x correction (wvks) is accumulated via mm2b.
            vsc = const.tile([128, 8], bf16)
            for i in range(8):
                nc.vector.tensor_scalar_mul(
                    vsc[:, i:i + 1], v_sb[:], 1.0 if i in ACT_CHUNKS else 0.8
                )
            vks = const.tile([128, K * 32], bf16)
            nc.vector.memset(vks[:], 0.0)
            wvks = const.tile([128, K * 32], bf16)
            nc.vector.memset(wvks[:], 0.0)
            for j in range(4):
                nc.vector.tensor_copy(
                    vks[:, 32 * j : 32 * j + 129 * 7 + 1 : 129], vsc[:, 0:8]
                )
            for j in range(4):
                nc.vector.tensor_copy(
                    wvks[:, 32 * j : 32 * j + 129 * 7 + 1 : 129],
                    wv_sb[:, 0:1].broadcast_to([128, 8]),
                )

            nchunk = (K * TN) // 1024    # 8 mm1 chunks per tile (2 matmuls each)
            prev = None                  # state of tile t-1 awaiting aggregation

            def emit_softmax_head(st, subs=range(NSUB)):
                """TensorE transposes of exp-weights to [node, kperm] (both
                sub-tiles into one PSUM bank tile) + row sums over the 32 used
                columns via ScalarE accum_out. Emitted right after chunk 0."""
                w_sb = st["w_sb"]
                wT_ps = wtps.tile([128, 256], bf16)
                st["wT_sb"], st["sums"] = [], []
                for s in subs:
                    wps = wT_ps[:, s * 128:(s + 1) * 128]
                    nc.tensor.transpose(
                        wps, w_sb[:, s * 128:(s + 1) * 128], mask[:]
                    )
                    wT_sb = smallp.tile([128, 32], bf16)
                    sums = smallp.tile([128, 1], f32)
                    # only columns 32j+i (i<8) carry scores; strided view.
                    # wT stays UNNORMALIZED (bf16) — the agg multiplies labels
                    # by raw exp-weights and 1/sum is applied once per node on
                    # the final [128, L] output, so nothing downstream waits
                    # on the normalization.
                    wT_used = wps.rearrange("p (j z) -> p j z", j=4)[:, :, 0:8]
                    nc.scalar.activation(
                        wT_sb[:].rearrange("p (j i) -> p j i", j=4),
                        wT_used, AF.Copy, accum_out=sums[:],
                    )
                    st["wT_sb"].append(wT_sb)
                    st["sums"].append(sums)

            def emit_softmax_tail(st, subs=range(NSUB)):
                """Per-node 1/sum (DVE), consumed only by the tiny final scale."""
                st["recip"] = []
                for s in subs:
                    recip = smallp.tile([128, 1], f32)
                    nc.vector.reciprocal(recip[:], st["sums"][s][:])
                    st["recip"].append(recip)
                st["out_sb"] = outp.tile([128, NSUB * L], bf16, name="out_sb", tag="out")

            def emit_agg(st, s):
                """Weighted label aggregation for sub-tile s on VectorE.
                Labels are l-major with kperm innermost, so the multiply
                (broadcast over outer l) runs in the DVE 16-bit 2x mode.
                The k-reduction is a 2-level pairwise tree (tensor_tensor adds
                at 2x) plus a short 8-wide tensor_reduce — cheaper than one
                32-wide reduce, which gets no 2x mode."""
                lab3 = st["lab_sb"][:, s * L * K:(s + 1) * L * K].rearrange(
                    "p (l k) -> p l k", k=K
                )
                al3 = st["wT_sb"][s][:, 0:K].rearrange(
                    "p (o k) -> p o k", o=1
                ).broadcast_to([128, L, K])
                prod = dkp.tile([128, L * K], bf16, name=f"prod{s}", tag=f"prod{s}")
                p3 = prod[:].rearrange("p (l k) -> p l k", k=K)
                nc.vector.tensor_tensor(p3, lab3, al3, OP.mult)
                t1 = dkp.tile([128, L * 16], bf16, name=f"t1_{s}", tag=f"t1_{s}")
                t13 = t1[:].rearrange("p (l k) -> p l k", k=16)
                with nc.allow_low_precision(reason="bf16 partials; final f32 gate is 2e-2"):
                    nc.vector.tensor_tensor(
                        t13, p3[:, :, 0:16], p3[:, :, 16:32], OP.add
                    )
                    t2 = dkp.tile([128, L * 8], bf16, name=f"t2_{s}", tag=f"t2_{s}")
                    t23 = t2[:].rearrange("p (l k) -> p l k", k=8)
                    nc.vector.tensor_tensor(
                        t23, t13[:, :, 0:8], t13[:, :, 8:16], OP.add
                    )
                    u = dkp.tile([128, L], bf16, name=f"u{s}", tag=f"u{s}")
                    nc.vector.tensor_reduce(
                        u[:], t23, op=OP.add, axis=mybir.AxisListType.X,
                    )
                    # softmax normalization: one per-node scale on [128, L]
                    nc.vector.tensor_scalar_mul(
                        st["out_sb"][:, s * L:(s + 1) * L], u[:],
                        st["recip"][s][:, 0:1],
                    )

            def emit_agg_finish(st):
                nc.sync.dma_start(out_ext[st["t"]], st["out_sb"][:])

            for t in range(nt):
                x_sb = xp.tile([128, K * TN], f8)
                if t == 0:
                    # quarter the first x load so chunk 0's matmuls start
                    # after ~256KB instead of a full 1MB (shortens the ramp)
                    q = K * TN // 4
                    for qi in range(4):
                        nc.sync.dma_start(
                            x_sb[:, qi * q:(qi + 1) * q], x_ext[t][:, qi * q:(qi + 1) * q]
                        )
                else:
                    nc.sync.dma_start(x_sb[:], x_ext[t][:])
                lab_sb = labp.tile([128, NSUB * L * K], bf16)
                nc.sync.dma_start(lab_sb[:], lab_ext[t][:])

                h_sb = hp.tile([128, K * TN], bf16)
                s_ps = sps.tile([128, TN], f32, name="s_ps", tag="sps")

                # the last tile's second sub-tile (nodes 6272..6399) is all
                # padding, so its scores are never consumed: run the score
                # matmuls and e/exp at half width there
                tw = 128 if t == nt - 1 else TN

                def emit_mm2(k):
                    j, i = k % 4, k // 4
                    last_k = k >= K - 4
                    sj = s_ps[32 * j:32 * (j + 1), 0:tw]
                    nc.tensor.matmul(
                        sj, vks[:, k * 32:(k + 1) * 32],
                        h_sb[:, k * TN:k * TN + tw],
                        start=(k < 4),
                        stop=(last_k and i not in DVE_CHUNKS),
                        tile_position=(0, 32 * j),
                    )
                    if i in DVE_CHUNKS:
                        # exact lrelu correction for the relu-only DVE chunks:
                        # += 0.2*(W@v)^T x  (x is already in SBUF)
                        nc.tensor.matmul(
                            sj, wvks[:, k * 32:(k + 1) * 32],
                            x_sb[:, k * TN:k * TN + tw],
                            start=False, stop=last_k,
                            tile_position=(0, 32 * j),
                        )

                def emit_chunk(c):
                    # 1024-col chunk: two 512-col matmuls, then one lrelu
                    # PSUM->SBUF move on ScalarE (full Prelu) or VectorE
                    # (relu-only; the 0.2z part is restored in mm2)
                    z_ps = zps.tile([128, 1024], f32, name="z_ps", tag="z")
                    nc.tensor.matmul(
                        z_ps[:, 0:512], W_sb[:], x_sb[:, c * 1024:c * 1024 + 512]
                    )
                    nc.tensor.matmul(
                        z_ps[:, 512:1024], W_sb[:], x_sb[:, c * 1024 + 512:(c + 1) * 1024]
                    )
                    hc = h_sb[:, c * 1024:(c + 1) * 1024]
                    if t == nt - 1:
                        # last tile: sub-1 columns (per-k n>=128) feed the
                        # all-padding sub-tile and the half-width mm2 never
                        # reads them — activate only the first 128 n per k
                        zv = z_ps[:].rearrange("p (k n) -> p k n", k=4)[:, :, 0:128]
                        hv = hc.rearrange("p (k n) -> p k n", k=4)[:, :, 0:128]
                        if c in ACT_CHUNKS:
                            nc.scalar.activation(hv, zv, AF.Prelu, alpha=NEG)
                        else:
                            nc.vector.tensor_scalar_max(hv, zv, 0.0)
                    elif c in ACT_CHUNKS:
                        nc.scalar.activation(hc, z_ps[:], AF.Prelu, alpha=NEG)
                    else:
                        nc.vector.tensor_scalar_max(hc, z_ps[:], 0.0)

                emit_chunk(0)
                if prev is not None:
                    emit_softmax_head(prev)
                # mm2 runs one chunk behind mm1 (chunk c-1's lrelu output is
                # ready), so the scores finish right after the last chunk and
                # e/exp don't stall the ACT queue for a full mm2 tail.
                for c in range(1, nchunk):
                    emit_chunk(c)
                    for k in range(4 * (c - 1), 4 * c):
                        emit_mm2(k)
                    if prev is not None and c == 1:
                        emit_softmax_tail(prev)
                    if prev is not None and c == 2:
                        emit_agg(prev, 0)
                    if prev is not None and c == 5:
                        emit_agg(prev, 1)
                if prev is not None:
                    emit_agg_finish(prev)
                for k in range(4 * (nchunk - 1), K):
                    emit_mm2(k)

                e_sb = wp.tile([128, TN], f32)
                nc.scalar.activation(
                    e_sb[:, 0:tw], s_ps[0:128, 0:tw], AF.Prelu,
                    bias=bias_sb[:, 0:1], alpha=NEG,
                )
                w_sb = wp.tile([128, TN], bf16)
                nc.scalar.activation(w_sb[:, 0:tw], e_sb[:, 0:tw], AF.Exp)

                prev = {"t": t, "w_sb": w_sb, "lab_sb": lab_sb}

            # drain the last tile. Its second sub-tile (nodes 6272..6399)
            # is entirely zero-padding discarded by unshard, so its whole
            # tail chain is skipped; the untouched half of the output stays
            # zero (PJRT output buffers are donated pre-zeroed).
            emit_softmax_head(prev, subs=(0,))
            emit_softmax_tail(prev, subs=(0,))
            emit_agg(prev, 0)
            nc.sync.dma_start(
                out_ext[prev["t"]][:, 0:L], prev["out_sb"][:, 0:L]
            )
    nc.compile()
    return nc


def shard_inputs(x, lab, nt=NT, nper=NPER):
    import ml_dtypes

    bf16 = ml_dtypes.bfloat16
    f8 = ml_dtypes.float8_e3m4
    npad = nt * TN
    xs = np.zeros((npad, K, D), f8)
    xs[:nper] = x.astype(f8)
    ls = np.zeros((npad, K, L), bf16)
    ls[:nper] = lab.astype(bf16)
    xf = np.ascontiguousarray(
        xs.reshape(nt, TN, K, D).transpose(0, 3, 2, 1)
    ).reshape(nt, 128, K * TN)
    # [t, s, p, k, l] -> k-permute -> [t, p, s, l, k']
    ls5 = ls.reshape(nt, NSUB, 128, K, L)[:, :, :, KPERM, :]
    lf = np.ascontiguousarray(ls5.transpose(0, 2, 1, 4, 3)).reshape(
        nt, 128, NSUB * L * K
    )
    return xf, lf


def unshard_output(o, nt=NT, nper=NPER):
    # o[t, p, s*L + l] = pred[node = t*TN + s*128 + p, l]   (bf16)
    return (
        o.astype(np.float32)
        .reshape(nt, 128, NSUB, L)
        .transpose(0, 2, 1, 3)
        .reshape(nt * TN, L)[:nper]
    )


def make_in_maps(inputs):
    x = np.asarray(inputs["para_neighbors"], np.float32)
    lab = np.asarray(inputs["para_nei_labels"], np.float32)
    Wm = np.ascontiguousarray(np.asarray(inputs["linear"], np.float32))
    v = np.ascontiguousarray(np.asarray(inputs["e_vec"], np.float32))
    b = np.asarray(inputs["bias"], np.float32).reshape(K)
    # bias128[32j+i] = b[4i+j] (rows 32j+i, i<8 are the used score rows)
    b128 = np.zeros((128, 1), np.float32)
    for jj in range(4):
        for ii in range(8):
            b128[32 * jj + ii, 0] = b[4 * ii + jj]
    # 0.2*(W@v) from the bf16-rounded W so the correction matches the PE's z
    import ml_dtypes

    Wb = Wm.astype(ml_dtypes.bfloat16).astype(np.float32)
    vb = v.astype(ml_dtypes.bfloat16).astype(np.float32)
    wv = np.ascontiguousarray(NEG * (Wb @ vb)).astype(ml_dtypes.bfloat16)
    W16 = np.ascontiguousarray(Wm.astype(ml_dtypes.bfloat16))
    v16 = np.ascontiguousarray(v.astype(ml_dtypes.bfloat16))
    in_maps = []
    for i in range(NCORES):
        xf, lf = shard_inputs(x[i * NPER:(i + 1) * NPER], lab[i * NPER:(i + 1) * NPER])
        in_maps.append({"x": xf, "lab": lf, "w": W16, "v": v16, "b": b128, "wv": wv})
    return in_maps


def kernel(para_neighbors, para_nei_labels, linear, e_vec, bias):
    from concourse.bass_utils import run_bass_kernel_spmd

    global LAST_RESULT
    if "nc" not in _cache:
        _cache["nc"] = build(NT)
    nc = _cache["nc"]

    in_maps = make_in_maps({
        "para_neighbors": para_neighbors, "para_nei_labels": para_nei_labels,
        "linear": linear, "e_vec": e_vec, "bias": bias,
    })
    res = run_bass_kernel_spmd(nc, in_maps, core_ids=list(range(NCORES)))
    LAST_RESULT = res
    outs = [unshard_output(res.results[i]["out"]) for i in range(NCORES)]
    return np.ascontiguousarray(np.concatenate(outs, axis=0))
